# revision 4
# baseline (speedup 1.0000x reference)
"""Sliding-window causal attention (window=256) on 8 TRN2 NeuronCores.

Sharding: batch x head-group. Core c handles batch b = c//4 and heads
4g..4g+3 where g = c%4 (columns 256g:256g+256 of Wq/Wk/Wv, rows of Wo).
Each core computes its 4 heads' attention and a partial output projection
y_partial = attn_out_heads @ Wo[256g:256g+256, :]; the host sums the 4
partials per batch and adds bo.

Per-core kernel (all sizes hardcoded: S=2048, D=1024, HD=64, W=256):
  1. xT = x.T via PE transposes (fp32 needs matmul-based transpose).
  2. qT/kT per head-pair ([128,2048], d on partitions), V for all 4 heads
     ([s,4*64+ones]) -- fp32r matmuls, contraction over D.
  3. Per head, per key-block kb: transposed scores S_T[k,q] = K_b @ Q_win^T
     (fp32r, free dim = 384 q-window), exp on ACT (scale=1/8 fused),
     band-mask multiply (bf16), PV matmuls accumulate O[q, 64+1] in PSUM
     with an appended ones column of V producing the softmax denominator.
  4. Normalize O by reciprocal of the l column (per-partition broadcast),
     PE-transpose O -> O_T, output projection vs bf16 Wo.
No softmax max-subtraction: scores ~ N(0,1), |s| < ~8, exp is safe in f32.
"""

import sys

for _p in ("/opt/trn_rl_repo", "/opt/pypackages"):
    if _p not in sys.path:
        sys.path.append(_p)

from contextlib import ExitStack

import numpy as np

import concourse.bass as bass
import concourse.mybir as mybir
import concourse.tile as tile
from concourse.bass_utils import run_bass_kernel_spmd
from concourse.masks import make_identity

F32 = mybir.dt.float32
F32R = mybir.dt.float32r
BF16 = mybir.dt.bfloat16

B, S, D = 2, 2048, 1024
H, HD, WIN = 16, 64, 256
G = 4          # heads per core
GD = G * HD    # 256 weight cols per core
N_CORES = 8
P = 128
SB = S // P    # 16 s-blocks
KT = D // P    # 8 contraction tiles
SCALE = 1.0 / np.sqrt(HD)


def r(ap):
    """View an fp32 AP as float32r for full-rate PE matmul."""
    return ap.bitcast(F32R)


def build_kernel(dma_psum_out: bool = False):
    nc = bass.Bass(
        "TRN2",
        target_bir_lowering=False,
        debug=False,
        enable_asserts=False,
        num_devices=N_CORES,
    )
    x = nc.dram_tensor("x", [S, D], F32, kind="ExternalInput").ap()
    wq = nc.dram_tensor("wq", [D, GD], F32, kind="ExternalInput").ap()
    wk = nc.dram_tensor("wk", [D, GD], F32, kind="ExternalInput").ap()
    wv = nc.dram_tensor("wv", [D, GD], F32, kind="ExternalInput").ap()
    wo = nc.dram_tensor("wo", [GD, D], F32, kind="ExternalInput").ap()
    bq = nc.dram_tensor("bq", [GD], F32, kind="ExternalInput").ap()
    bk = nc.dram_tensor("bk", [GD], F32, kind="ExternalInput").ap()
    bv = nc.dram_tensor("bv", [GD], F32, kind="ExternalInput").ap()
    y = nc.dram_tensor("y", [S, D], F32, kind="ExternalOutput").ap()

    with tile.TileContext(nc) as tc, ExitStack() as ctx:
        const = ctx.enter_context(tc.tile_pool(name="const", bufs=1))
        wpool = ctx.enter_context(tc.tile_pool(name="weights", bufs=1))
        big = ctx.enter_context(tc.tile_pool(name="big", bufs=1))
        xload = ctx.enter_context(tc.tile_pool(name="xload", bufs=6))
        qkpool = ctx.enter_context(tc.tile_pool(name="qk", bufs=2))
        ptpool = ctx.enter_context(tc.tile_pool(name="pt", bufs=6))
        opool = ctx.enter_context(tc.tile_pool(name="osb", bufs=2))
        otpool = ctx.enter_context(tc.tile_pool(name="ot", bufs=2))
        rlpool = ctx.enter_context(tc.tile_pool(name="rl", bufs=2))
        ysb = ctx.enter_context(tc.tile_pool(name="ysb", bufs=3))
        psx = ctx.enter_context(tc.tile_pool(name="psx", bufs=2, space="PSUM"))
        ps512 = ctx.enter_context(tc.tile_pool(name="ps512", bufs=3, space="PSUM"))
        pso = ctx.enter_context(tc.tile_pool(name="pso", bufs=1, space="PSUM"))

        # ---- constants ----
        ident_r = const.tile([P, P], F32R)
        make_identity(nc, ident_r)
        ident_b = const.tile([P, P], BF16)
        make_identity(nc, ident_b)
        # band masks: panel 0 (diag block): keep j >= c; panel 1 (oldest
        # block): keep j < c.  (j = local q col, c = local k partition)
        maskp = const.tile([P, 2, P], BF16)
        nc.gpsimd.memset(maskp[:], 1.0)
        nc.gpsimd.affine_select(
            out=maskp[:, 0], in_=maskp[:, 0],
            compare_op=mybir.AluOpType.is_ge, fill=0.0,
            base=0, pattern=[[1, P]], channel_multiplier=-1,
        )
        nc.gpsimd.affine_select(
            out=maskp[:, 1], in_=maskp[:, 1],
            compare_op=mybir.AluOpType.is_ge, fill=0.0,
            base=-1, pattern=[[-1, P]], channel_multiplier=1,
        )
        ones1 = const.tile([1, P], F32)
        nc.gpsimd.memset(ones1[:], 1.0)

        bqs = const.tile([P, 2], F32)
        nc.sync.dma_start(bqs[:], bq.rearrange("(o p) -> p o", p=P))
        bks = const.tile([P, 2], F32)
        nc.sync.dma_start(bks[:], bk.rearrange("(o p) -> p o", p=P))
        bvs = const.tile([1, GD], F32)
        nc.sync.dma_start(bvs[:], bv[None, :])

        wq_sb = wpool.tile([P, KT, GD], F32)
        nc.sync.dma_start(wq_sb[:], wq.rearrange("(o p) n -> p o n", p=P))
        wk_sb = wpool.tile([P, KT, GD], F32)
        nc.sync.dma_start(wk_sb[:], wk.rearrange("(o p) n -> p o n", p=P))
        wv_sb = wpool.tile([P, KT, GD], F32)
        nc.sync.dma_start(wv_sb[:], wv.rearrange("(o p) n -> p o n", p=P))
        wo_f = wpool.tile([P, 2, D], F32)
        nc.sync.dma_start(wo_f[:], wo.rearrange("(o p) n -> p o n", p=P))
        wo_b = wpool.tile([P, 2, D], BF16)
        nc.vector.tensor_copy(out=wo_b[:], in_=wo_f[:])

        # ---- phase 1: xT [d-part, kt, s] ----
        xT = big.tile([P, KT, S], F32)
        for tg in range(SB // 4):
            xs = []
            for i in range(4):
                t = 4 * tg + i
                xt_ = xload.tile([P, D], F32, tag="xs")
                nc.sync.dma_start(xt_[:], x[P * t : P * (t + 1), :])
                xs.append(xt_)
            for db in range(KT):
                ps = psx.tile([P, 512], F32R)
                for i in range(4):
                    nc.tensor.transpose(
                        ps[:, P * i : P * (i + 1)],
                        r(xs[i][:, P * db : P * (db + 1)]),
                        ident_r[:],
                    )
                nc.vector.tensor_copy(
                    out=xT[:, db, 512 * tg : 512 * (tg + 1)],
                    in_=ps[:].bitcast(F32),
                )

        # ---- phase 2a: V1 [s-part, sb, head, 64+1] (bf16, ones col) ----
        v1 = big.tile([P, SB, G, HD + 1], BF16)
        nc.gpsimd.memset(v1[:, :, :, HD], 1.0)
        for t in range(SB):
            ps = ps512.tile([P, 512], F32, tag="ps512")
            pv = ps[:, :GD]
            nc.tensor.matmul(pv, r(ones1[:]), r(bvs[:]), start=True, stop=False)
            for kt in range(KT):
                nc.tensor.matmul(
                    pv,
                    r(xT[:, kt, P * t : P * (t + 1)]),
                    r(wv_sb[:, kt, :]),
                    start=False,
                    stop=(kt == KT - 1),
                )
            nc.vector.tensor_copy(
                out=v1[:, t, :, 0:HD],
                in_=pv.rearrange("p (h d) -> p h d", d=HD),
            )

        # ---- phases 2b/3: per head-pair projections + attention ----
        ot_tiles = []
        for p_ in range(2):
            # qT/kT for the pair: [128 (2 heads x 64 d), S] fp32
            qT = qkpool.tile([P, S], F32, tag="qT")
            kTt = qkpool.tile([P, S], F32, tag="kT")
            for w_sb, bias, dst in ((wq_sb, bqs, qT), (wk_sb, bks, kTt)):
                for ch in range(4):
                    ps = ps512.tile([P, 512], F32, tag="ps512")
                    for kt in range(KT):
                        nc.tensor.matmul(
                            ps[:],
                            r(w_sb[:, kt, P * p_ : P * (p_ + 1)]),
                            r(xT[:, kt, 512 * ch : 512 * (ch + 1)]),
                            start=(kt == 0),
                            stop=(kt == KT - 1),
                        )
                    # psum -> sbuf with per-partition bias add, on ACT
                    nc.scalar.add(
                        out=dst[:, 512 * ch : 512 * (ch + 1)],
                        in_=ps[:],
                        add=bias[:, p_ : p_ + 1],
                    )

            o_sb = opool.tile([P, SB, 2, HD], BF16, tag="osb")
            for hh in range(2):
                hp = 2 * p_ + hh  # head index within core (0..3)
                qh = qT[HD * hh : HD * (hh + 1), :]
                kh = kTt[HD * hh : HD * (hh + 1), :]
                pts = {}
                for half in range(2):
                    HB = SB // 2
                    pso_t = pso.tile([P, HB, P], F32, tag="pso")
                    for qq in range(HB):
                        qb = half * HB + qq
                        # produce P_T tile for key-block kb = qb
                        kb = qb
                        w = min(384, S - P * kb)
                        pss = ps512.tile([P, 512], F32, tag="ps512")
                        nc.tensor.matmul(
                            pss[:, :w],
                            r(kh[:, P * kb : P * (kb + 1)]),
                            r(qh[:, P * kb : P * kb + w]),
                            start=True,
                            stop=True,
                        )
                        pt = ptpool.tile([P, 384], BF16, tag="pt")
                        pts[kb] = pt
                        nc.scalar.activation(
                            out=pt[:, :w], in_=pss[:, :w],
                            func=mybir.ActivationFunctionType.Exp,
                            scale=float(SCALE),
                        )
                        nc.vector.tensor_mul(
                            out=pt[:, 0:P], in0=pt[:, 0:P], in1=maskp[:, 0]
                        )
                        if w > 2 * P:
                            nc.vector.tensor_mul(
                                out=pt[:, 2 * P : 3 * P],
                                in0=pt[:, 2 * P : 3 * P],
                                in1=maskp[:, 1],
                            )
                        # consume: the 3-matmul accumulation group for this qb
                        first_kb = max(0, qb - 2)
                        for kb2 in range(first_kb, qb + 1):
                            off = (qb - kb2) * P
                            nc.tensor.matmul(
                                pso_t[:, qq, 0 : HD + 1],
                                pts[kb2][:, off : off + P],
                                v1[:, kb2, hp, :],
                                start=(kb2 == first_kb),
                                stop=(kb2 == qb),
                            )
                        pts.pop(qb - 2, None)
                    # epilogue: normalize by the accumulated l column
                    rl = rlpool.tile([P, HB, 1], F32, tag="rl")
                    nc.vector.reciprocal(out=rl[:], in_=pso_t[:, :, HD : HD + 1])
                    nc.vector.tensor_mul(
                        out=o_sb[:, half * HB : (half + 1) * HB, hh, :],
                        in0=pso_t[:, :, 0:HD],
                        in1=rl[:].to_broadcast([P, HB, HD]),
                    )

            # transpose O -> O_T [hd (2 heads), s] bf16
            ot = otpool.tile([P, S], BF16, tag="ot")
            ot_tiles.append(ot)
            for t4 in range(SB // 4):
                pst = ps512.tile([P, 512], F32, tag="ps512")
                for qq in range(4):
                    qb = 4 * t4 + qq
                    for hh in range(2):
                        nc.tensor.matmul(
                            pst[HD * hh : HD * (hh + 1), P * qq : P * (qq + 1)],
                            o_sb[:, qb, hh, :],
                            ident_b[:],
                            start=True,
                            stop=True,
                        )
                nc.vector.tensor_copy(
                    out=ot[:, 512 * t4 : 512 * (t4 + 1)], in_=pst[:]
                )

        # ---- phase 4: output projection ----
        for t in range(SB):
            for nch in range(2):
                psy = ps512.tile([P, 512], F32, tag="ps512")
                for p_ in range(2):
                    nc.tensor.matmul(
                        psy[:],
                        ot_tiles[p_][:, P * t : P * (t + 1)],
                        wo_b[:, p_, 512 * nch : 512 * (nch + 1)],
                        start=(p_ == 0),
                        stop=(p_ == 1),
                    )
                dst = y[P * t : P * (t + 1), 512 * nch : 512 * (nch + 1)]
                if dma_psum_out:
                    nc.sync.dma_start(dst, psy[:])
                else:
                    yt = ysb.tile([P, 512], F32, tag="ysb")
                    nc.scalar.copy(out=yt[:], in_=psy[:])
                    nc.sync.dma_start(dst, yt[:])

    return nc


_NC = None


def _get_nc():
    global _NC
    if _NC is None:
        _NC = build_kernel()
    return _NC


def make_in_maps(x, Wq, bq, Wk, bk, Wv, bv, Wo, bo):
    in_maps = []
    for c in range(N_CORES):
        b, g = divmod(c, 4)
        cs = slice(GD * g, GD * (g + 1))
        in_maps.append(
            {
                "x": np.ascontiguousarray(x[b], dtype=np.float32),
                "wq": np.ascontiguousarray(Wq[:, cs], dtype=np.float32),
                "wk": np.ascontiguousarray(Wk[:, cs], dtype=np.float32),
                "wv": np.ascontiguousarray(Wv[:, cs], dtype=np.float32),
                "wo": np.ascontiguousarray(Wo[cs, :], dtype=np.float32),
                "bq": np.ascontiguousarray(bq[cs], dtype=np.float32),
                "bk": np.ascontiguousarray(bk[cs], dtype=np.float32),
                "bv": np.ascontiguousarray(bv[cs], dtype=np.float32),
            }
        )
    return in_maps


def combine_outputs(results, bo):
    out = np.zeros((B, S, D), dtype=np.float32)
    for c in range(N_CORES):
        out[c // 4] += results[c]["y"]
    out += bo.astype(np.float32)
    return out


def kernel(x, Wq, bq, Wk, bk, Wv, bv, Wo, bo):
    nc = _get_nc()
    in_maps = make_in_maps(x, Wq, bq, Wk, bk, Wv, bv, Wo, bo)
    res = run_bass_kernel_spmd(nc, in_maps, core_ids=list(range(N_CORES)))
    return combine_outputs(res.results, np.asarray(bo))
